# revision 6
# baseline (speedup 1.0000x reference)
"""AttentionBlock kernel for Trainium2 (8 NeuronCores, batch-sharded).

Computes, per sample b:
    q = Wq @ x + bq            [32, N]
    k = Wk @ x + bk            [32, N]
    v = Wv @ x + bv            [256, N]
    attn = softmax(q^T k)      [N, N] (softmax over keys)
    out = gamma * (v @ attn^T) + x

Layout trick: everything is computed transpose-free by working with
S^T [keys, queries]: the softmax denominator (a partition-dim sum) is
obtained with a ones-matrix matmul accumulated alongside the PV matmul,
and normalization is deferred to the [256, N] output (N*C elements
instead of N^2).  Matmul operands use float32r (1 cycle/row on the PE
for free dims >= 256, vs 4 for plain fp32).
"""

from contextlib import ExitStack

import numpy as np

import concourse.bass as bass
import concourse.mybir as mybir
import concourse.tile as tile
from concourse import bacc
from concourse.bass_utils import run_bass_kernel_spmd

B, C, H, W = 8, 256, 64, 64
N = H * W        # 4096
D = 32           # C // 8
NCORES = 8
P = 128
F32 = mybir.dt.float32
F32R = mybir.dt.float32r

NW = 8           # n-chunks of 512 queries
NCH = N // NW    # 512
MP = N // P      # 32 key-chunks of 128
PAIR = 2         # key-chunks per exp group
NG = MP // PAIR  # 16 groups


def build_bass():
    nc = bacc.Bacc("TRN2", target_bir_lowering=False, debug=False,
                   enable_asserts=False, num_devices=NCORES)

    x_d = nc.dram_tensor("x", [C, N], F32R, kind="ExternalInput").ap()
    wqT_d = nc.dram_tensor("wqT", [C, D], F32R, kind="ExternalInput").ap()
    wkT_d = nc.dram_tensor("wkT", [C, D], F32R, kind="ExternalInput").ap()
    wvT_d = nc.dram_tensor("wvT", [C, C], F32R, kind="ExternalInput").ap()
    bq_d = nc.dram_tensor("bq", [D, 1], F32, kind="ExternalInput").ap()
    bk_d = nc.dram_tensor("bk", [D, 1], F32, kind="ExternalInput").ap()
    bvb_d = nc.dram_tensor("bvb", [P, C], F32, kind="ExternalInput").ap()
    gam_d = nc.dram_tensor("gam", [P, 1], F32, kind="ExternalInput").ap()
    ones_d = nc.dram_tensor("ones", [P, P], F32R, kind="ExternalInput").ap()
    out_d = nc.dram_tensor("out", [C, N], F32, kind="ExternalOutput").ap()

    with tile.TileContext(nc) as tc, ExitStack() as ctx:
        const = ctx.enter_context(tc.tile_pool(name="const", bufs=1))
        xp = ctx.enter_context(tc.tile_pool(name="xp", bufs=1))
        qk = ctx.enter_context(tc.tile_pool(name="qk", bufs=1))
        vt = ctx.enter_context(tc.tile_pool(name="vt", bufs=1))
        pt = ctx.enter_context(tc.tile_pool(name="pt", bufs=3))
        op = ctx.enter_context(tc.tile_pool(name="op", bufs=2))
        ps_st = ctx.enter_context(tc.tile_pool(name="ps_st", bufs=2, space="PSUM"))
        ps_out = ctx.enter_context(tc.tile_pool(name="ps_out", bufs=1, space="PSUM"))
        ps_den = ctx.enter_context(tc.tile_pool(name="ps_den", bufs=1, space="PSUM"))
        ps_pro = ctx.enter_context(tc.tile_pool(name="ps_pro", bufs=1, space="PSUM"))

        # ---- load inputs ----
        x_sb = xp.tile([P, 2, N], F32R)           # [128, c-half, 4096]
        nc.sync.dma_start(out=x_sb[:, 0, :], in_=x_d[0:P, :])
        nc.sync.dma_start(out=x_sb[:, 1, :], in_=x_d[P:C, :])

        wqT_sb = const.tile([P, 2, D], F32R)
        nc.sync.dma_start(out=wqT_sb[:, 0, :], in_=wqT_d[0:P, :])
        nc.sync.dma_start(out=wqT_sb[:, 1, :], in_=wqT_d[P:C, :])
        wkT_sb = const.tile([P, 2, D], F32R)
        nc.sync.dma_start(out=wkT_sb[:, 0, :], in_=wkT_d[0:P, :])
        nc.sync.dma_start(out=wkT_sb[:, 1, :], in_=wkT_d[P:C, :])
        wvT_sb = const.tile([P, 2, C], F32R)
        nc.sync.dma_start(out=wvT_sb[:, 0, :], in_=wvT_d[0:P, :])
        nc.sync.dma_start(out=wvT_sb[:, 1, :], in_=wvT_d[P:C, :])
        bq_sb = const.tile([D, 1], F32)
        nc.sync.dma_start(out=bq_sb, in_=bq_d)
        bk_sb = const.tile([D, 1], F32)
        nc.sync.dma_start(out=bk_sb, in_=bk_d)
        bvb_sb = const.tile([P, C], F32)
        nc.sync.dma_start(out=bvb_sb, in_=bvb_d)
        gam_sb = const.tile([P, 1], F32)
        nc.sync.dma_start(out=gam_sb, in_=gam_d)
        ones_sb = const.tile([P, P], F32R)
        nc.sync.dma_start(out=ones_sb, in_=ones_d)

        # ---- prologue: q, k [32, N]; v^T as 32 chunks [128m, 256d] ----
        q_sb = qk.tile([D, N], F32R)
        k_sb = qk.tile([D, N], F32R)
        for j in range(NW):
            sl = slice(j * NCH, (j + 1) * NCH)
            ps_q = ps_pro.tile([D, NCH], F32, tag="pro")
            for ci in range(2):
                nc.tensor.matmul(ps_q, lhsT=wqT_sb[:, ci, :],
                                 rhs=x_sb[:, ci, sl],
                                 start=(ci == 0), stop=(ci == 1))
            nc.vector.tensor_scalar_add(out=q_sb[:, sl], in0=ps_q, scalar1=bq_sb)
            ps_k = ps_pro.tile([D, NCH], F32, tag="pro")
            for ci in range(2):
                nc.tensor.matmul(ps_k, lhsT=wkT_sb[:, ci, :],
                                 rhs=x_sb[:, ci, sl],
                                 start=(ci == 0), stop=(ci == 1))
            nc.vector.tensor_scalar_add(out=k_sb[:, sl], in0=ps_k, scalar1=bk_sb)

        vT_sb = vt.tile([P, MP, C], F32R)         # [128, m-chunk, 256]
        for m in range(MP):
            msl = slice(m * P, (m + 1) * P)
            ps_v = ps_pro.tile([P, C], F32, tag="pro")
            for ci in range(2):
                nc.tensor.matmul(ps_v, lhsT=x_sb[:, ci, msl],
                                 rhs=wvT_sb[:, ci, :],
                                 start=(ci == 0), stop=(ci == 1))
            nc.vector.tensor_add(out=vT_sb[:, m, :], in0=ps_v, in1=bvb_sb)

        # ---- main attention loop ----
        for n in range(NW):
            nsl = slice(n * NCH, (n + 1) * NCH)
            out_ps = ps_out.tile([P, 2, NCH], F32)     # 2 banks
            den_ps = ps_den.tile([P, NCH], F32)        # 1 bank
            for g in range(NG):
                st_ps = ps_st.tile([P, PAIR, NCH], F32)   # 2 banks, dbl-buf
                for c2 in range(PAIR):
                    m = g * PAIR + c2
                    nc.tensor.matmul(st_ps[:, c2, :],
                                     lhsT=k_sb[:, m * P:(m + 1) * P],
                                     rhs=q_sb[:, nsl],
                                     start=True, stop=True)
                p_sb = pt.tile([P, PAIR, NCH], F32R)
                nc.scalar.activation(out=p_sb, in_=st_ps,
                                     func=mybir.ActivationFunctionType.Exp)
                for c2 in range(PAIR):
                    m = g * PAIR + c2
                    first = (g == 0 and c2 == 0)
                    last = (g == NG - 1 and c2 == PAIR - 1)
                    prhs = p_sb[:, c2, :]
                    nc.tensor.matmul(out_ps[:, 0, :],
                                     lhsT=vT_sb[:, m, 0:P], rhs=prhs,
                                     start=first, stop=last)
                    nc.tensor.matmul(out_ps[:, 1, :],
                                     lhsT=vT_sb[:, m, P:C], rhs=prhs,
                                     start=first, stop=last)
                    nc.tensor.matmul(den_ps,
                                     lhsT=ones_sb, rhs=prhs,
                                     start=first, stop=last)
            # normalize: out = gamma * num / den + x
            rd_sb = op.tile([P, NCH], F32)
            nc.vector.reciprocal_approx_fast(out=rd_sb, in_=den_ps)
            nc.vector.tensor_scalar_mul(out=rd_sb, in0=rd_sb, scalar1=gam_sb)
            out_sb = op.tile([P, 2, NCH], F32)
            for hh in range(2):
                nc.vector.tensor_mul(out=out_sb[:, hh, :],
                                     in0=out_ps[:, hh, :], in1=rd_sb)
                nc.vector.tensor_add(out=out_sb[:, hh, :],
                                     in0=out_sb[:, hh, :],
                                     in1=x_sb[:, hh, nsl].bitcast(F32))
                nc.sync.dma_start(out=out_d[hh * P:(hh + 1) * P, nsl],
                                  in_=out_sb[:, hh, :])
    nc.compile()
    return nc


_NC_CACHE = None


def _get_nc():
    global _NC_CACHE
    if _NC_CACHE is None:
        _NC_CACHE = build_bass()
    return _NC_CACHE


def _in_maps(inputs):
    x = np.ascontiguousarray(np.asarray(inputs["x"], dtype=np.float32))
    wqT = np.ascontiguousarray(np.asarray(inputs["Wq"], np.float32).T)
    wkT = np.ascontiguousarray(np.asarray(inputs["Wk"], np.float32).T)
    wvT = np.ascontiguousarray(np.asarray(inputs["Wv"], np.float32).T)
    bq = np.asarray(inputs["bq"], np.float32).reshape(D, 1).copy()
    bk = np.asarray(inputs["bk"], np.float32).reshape(D, 1).copy()
    bvb = np.ascontiguousarray(
        np.broadcast_to(np.asarray(inputs["bv"], np.float32)[None, :], (P, C)))
    gam = np.ascontiguousarray(
        np.broadcast_to(np.asarray(inputs["gamma"], np.float32).reshape(1, 1),
                        (P, 1)))
    maps = []
    for b in range(NCORES):
        maps.append({
            "x": np.ascontiguousarray(x[b].reshape(C, N)),
            "wqT": wqT, "wkT": wkT, "wvT": wvT,
            "bq": bq, "bk": bk, "bvb": bvb, "gam": gam,
            "ones": np.ones((P, P), np.float32),
        })
    return maps


def _run(inputs, **kw):
    nc = _get_nc()
    res = run_bass_kernel_spmd(nc, _in_maps(inputs), core_ids=list(range(NCORES)),
                               **kw)
    outs = [res.results[b]["out"].reshape(C, H, W) for b in range(NCORES)]
    return np.stack(outs, axis=0).astype(np.float32), res


def kernel(**inputs) -> np.ndarray:
    out, _ = _run(inputs)
    return out
